# revision 19
# baseline (speedup 1.0000x reference)
"""Multi-head attention (B=4, S=2048, H=1024, 16 heads) on 8 TRN2 NeuronCores.

Sharding: core c handles (batch b = c//2, head-group g = c%2 of 8 heads).
Per-core device program (activations feature-major / transposed):
  X^T  [1024,2048]  host-transposed bf16-cast query shard
  Q^T,K^T = Wq/k^T X^T            (PE, bf16, fp32 PSUM)
  V       = X W_v  (natural [s,d] via lhsT=X^T tiles), stored as
            V' = [1 | V] per head (leading ones column -> softmax sums)
  S^T  = per head, per k-tile: lhsT=K^T slice, rhs=Q^T slice
  E    = exp(S^T/8)  on ScalarE, bf16 out
  O'   = E^T V' per (head, q-block of 128): lhsT = E k-tile slices
         (LDWEIGHTS pipelines under the 65-wide matmuls: measured
         ~33ns/matmul vs 227ns for the [65, 512]-out orientation), so
         O'[q, 0] = softmax sum, O'[q, 1:65] = unnormalized O.
  normalize: per-partition reciprocal of col 0 + tensor_scalar_mul -> O[q,d]
  O^T  via PE transpose (identity matmul) -> assemble ot [d, q]
  Y^T  = W_o^T O^T   -> DMA out per [128,512] tile (split across 2 queues)
Host: gathers per-core Y^T tiles, sums the two head-group partials per batch,
adds b_o. b_qkv / attention bias are zero in this problem; nonzero values are
still handled (extra rank-1 bias matmuls / DVE bias adds) via build flags.

Scheduling: the PE is the critical engine; ScalarE exp (~285us) is a close
second. The AV phase of q-chunk qc is interleaved into the S/exp loop of the
NEXT q-chunk (one AV chain per k-tile iteration), and independent matmul
chains (V' tiles, next pair's Q/K projections, output-projection tiles) are
emitted as *filler* to keep the PE saturated while exp trails.
"""

import numpy as np
import ml_dtypes

import concourse.bass as bass
import concourse.tile as tile
from concourse import bacc
import concourse.mybir as mybir
from concourse.bass_utils import run_bass_kernel_spmd

F32 = mybir.dt.float32
BF16 = mybir.dt.bfloat16
AF = mybir.ActivationFunctionType

HIDDEN = 1024
HEADS = 16
HD = 64
B = 4
S_FULL = 2048
NCORES = 8
HPG = HEADS // 2          # heads per group/core = 8
GF = HPG * HD             # group feature width = 512
NPAIR = HPG // 2          # head pairs per core = 4


def build_program(S=S_FULL, has_bqkv=False, has_bias=False):
    KT = HIDDEN // 128            # hidden k-tiles = 8
    CH = min(512, S)              # free-dim chunk
    NQC = S // CH                 # q chunks
    SKT = S // 128                # seq k-tiles (attention contraction)
    NM = HIDDEN // 128            # output-projection m-tiles = 8
    NQB = CH // 128               # 128-row q-blocks per chunk = 4

    nc = bacc.Bacc(
        "TRN2",
        target_bir_lowering=False,
        debug=False,
        enable_asserts=False,
        num_devices=NCORES,
    )

    x_dram = nc.dram_tensor("x", [HIDDEN, S], BF16, kind="ExternalInput")  # X^T, host-transposed
    wqkv_dram = nc.dram_tensor("wqkv", [HIDDEN, 3 * GF], BF16, kind="ExternalInput")
    wo_dram = nc.dram_tensor("wo", [GF, HIDDEN], BF16, kind="ExternalInput")
    id_dram = nc.dram_tensor("ident", [128, 128], F32, kind="ExternalInput")
    if has_bqkv:
        bqkv_dram = nc.dram_tensor("bqkv", [1, 3 * GF], BF16, kind="ExternalInput")
    if has_bias:
        # host passes bias[0,0].T * 8 so exp(0.125*(S + bias8)) = exp(S/8 + bias)
        bias8_dram = nc.dram_tensor("bias8t", [S, S], F32, kind="ExternalInput")
    y_dram = nc.dram_tensor("y", [NM, NQC, 128, CH], BF16, kind="ExternalOutput")

    with tile.TileContext(nc) as tc:
        with (
            tc.tile_pool(name="res", bufs=1) as res,
            tc.tile_pool(name="wrk", bufs=2) as wrk,
            tc.tile_pool(name="ep", bufs=26) as ep,
            tc.tile_pool(name="ps", bufs=2, space="PSUM") as ps,
        ):
            xt = res.tile([128, KT * S], BF16, tag="xt")
            wqkv = res.tile([128, KT * 3 * GF], BF16, tag="wqkv")
            wo = res.tile([128, (GF // 128) * HIDDEN], BF16, tag="wo")
            ident = res.tile([128, 128], F32, tag="ident")
            # V' tiles: per s-tile block of 8 heads x 65 cols (col 0 = 1.0)
            vp = res.tile([128, SKT * HPG * 65], BF16, tag="vp")
            ot = res.tile([128, NPAIR * S], BF16, tag="ot")

            nc.vector.memset(vp[:, :], 1.0)

            # ---- input DMA, priority-ordered, wide rows only ----
            # First exp needs pair-0 Q/K chains: wqkv Q|K cols (2KB rows) and
            # xt qc0; then V cols (needed by V' chains inside the first
            # attention block), then the rest. Alternate sync/gpsimd queues.
            def eng_of(i):
                return nc.sync if (i % 2 == 0) else nc.gpsimd

            for kt in range(KT):
                eng_of(kt).dma_start(
                    wqkv[:, kt * 3 * GF: kt * 3 * GF + 2 * GF],
                    wqkv_dram[kt * 128:(kt + 1) * 128, 0:2 * GF],
                )
            for kt in range(KT):
                eng_of(kt).dma_start(
                    xt[:, kt * S: kt * S + CH],
                    x_dram[kt * 128:(kt + 1) * 128, 0:CH],
                )
            for kt in range(KT):
                eng_of(kt).dma_start(
                    wqkv[:, kt * 3 * GF + 2 * GF:(kt + 1) * 3 * GF],
                    wqkv_dram[kt * 128:(kt + 1) * 128, 2 * GF:],
                )
            nc.sync.dma_start(ident[:, :], id_dram[:, :])
            for qc in range(1, NQC):
                for kt in range(KT):
                    eng_of(kt).dma_start(
                        xt[:, kt * S + qc * CH: kt * S + (qc + 1) * CH],
                        x_dram[kt * 128:(kt + 1) * 128, qc * CH:(qc + 1) * CH],
                    )
            for ft in range(GF // 128):
                nc.gpsimd.dma_start(
                    wo[:, ft * HIDDEN:(ft + 1) * HIDDEN],
                    wo_dram[ft * 128:(ft + 1) * 128, :],
                )
            if has_bqkv:
                bq = res.tile([1, 3 * GF], BF16, tag="bq")
                nc.sync.dma_start(bq[:, :], bqkv_dram[:, :])
                ones = res.tile([1, CH], BF16, tag="ones")
                nc.vector.memset(ones[:, :], 1.0)

            def acc_matmul(out_ps, lhsT_of, rhs_of, bias_lhsT, bias_rhs):
                """Accumulate KT matmuls (+ optional rank-1 bias term) into PSUM."""
                if bias_lhsT is not None:
                    nc.tensor.matmul(out_ps, bias_lhsT, bias_rhs, start=True, stop=False)
                for kt in range(KT):
                    nc.tensor.matmul(
                        out_ps,
                        lhsT_of(kt),
                        rhs_of(kt),
                        start=(kt == 0 and bias_lhsT is None),
                        stop=(kt == KT - 1),
                    )

            # ---- chain emitters (each ~1.7us of PE time) ----
            def v_chain(st):
                """V' for all 8 heads at s-tile st, natural [s, d] layout."""
                vps = ps.tile([128, GF], F32, tag="acc", padded_shape=[128, 512])
                acc_matmul(
                    vps[:, 0:GF],
                    lambda kt, st=st: xt[:, kt * S + st * 128: kt * S + (st + 1) * 128],
                    lambda kt: wqkv[:, kt * 3 * GF + 2 * GF: kt * 3 * GF + 3 * GF],
                    ones[0:1, 0:128] if has_bqkv else None,
                    bq[0:1, 2 * GF:3 * GF] if has_bqkv else None,
                )
                dst = vp[:, st * HPG * 65:(st + 1) * HPG * 65]
                dst = dst.rearrange("p (h c) -> p h c", c=65)[:, :, 1:65]
                src = vps[:, 0:GF].rearrange("p (h c) -> p h c", c=64)
                nc.vector.tensor_copy(dst, src)

            qk_tiles = {}  # p -> (qt, ktt)

            def qk_chain(p, which, qc):
                """One [128, CH] chunk of Q^T (which=0) or K^T (which=1) for pair p."""
                if p not in qk_tiles:
                    qk_tiles[p] = (
                        wrk.tile([128, S], BF16, tag="qt", name=f"qt{p}"),
                        wrk.tile([128, S], BF16, tag="ktt", name=f"ktt{p}"),
                    )
                dst_sb = qk_tiles[p][which]
                colbase = p * 128 + (GF if which else 0)
                qkps = ps.tile([128, CH], F32, tag="acc")
                acc_matmul(
                    qkps[:, :],
                    lambda kt, cb=colbase: wqkv[:, kt * 3 * GF + cb: kt * 3 * GF + cb + 128],
                    lambda kt, qc=qc: xt[:, kt * S + qc * CH: kt * S + (qc + 1) * CH],
                    bq[0:1, colbase:colbase + 128] if has_bqkv else None,
                    ones[0:1, 0:CH] if has_bqkv else None,
                )
                nc.vector.tensor_copy(dst_sb[:, qc * CH:(qc + 1) * CH], qkps[:, :])

            def proj_chain(qc, m):
                """Y^T tile [128, CH] at (hidden-slice m, q-chunk qc)."""
                yps = ps.tile([128, CH], F32, tag="acc")
                for ft in range(GF // 128):
                    nc.tensor.matmul(
                        yps[:, :],
                        wo[:, ft * HIDDEN + m * 128: ft * HIDDEN + (m + 1) * 128],
                        ot[:, ft * S + qc * CH: ft * S + (qc + 1) * CH],
                        start=(ft == 0),
                        stop=(ft == GF // 128 - 1),
                    )
                ysb = wrk.tile([128, CH], BF16, tag="ysb", bufs=4)
                nc.vector.tensor_copy(ysb[:, :], yps[:, :])
                # split the writeback across queues so the last tiles don't
                # extend the tail; the final q-chunk lands after the last exp,
                # so its tiles may also use the then-idle scalar queue.
                if qc == NQC - 1:
                    third = CH // 4
                    nc.sync.dma_start(y_dram[m, qc, :, 0:third], ysb[:, 0:third])
                    nc.gpsimd.dma_start(
                        y_dram[m, qc, :, third:2 * third], ysb[:, third:2 * third]
                    )
                    nc.scalar.dma_start(y_dram[m, qc, :, 2 * third:], ysb[:, 2 * third:])
                else:
                    half = CH // 2
                    nc.sync.dma_start(y_dram[m, qc, :, 0:half], ysb[:, 0:half])
                    nc.gpsimd.dma_start(y_dram[m, qc, :, half:CH], ysb[:, half:CH])

            # FIFO of pending filler chains, popped inside the attention loop
            filler = []

            def emit_filler(n=1):
                for _ in range(n):
                    if not filler:
                        return
                    filler.pop(0)()

            def make_carry(p, qc, e_list):
                """Deferred AV phase for (p, qc): 8 av-units + 2 transpose
                tails, each ~0.5-0.6us of PE. Interleaved into the NEXT
                q-chunk's S/exp loop (one unit per k-tile iteration)."""
                units = []
                tps = []  # deferred (transpose + copy) closures

                def av_unit(j, qb, p=p, qc=qc, e_list=e_list, tps=tps):
                    h = p * 2 + j
                    avp = ps.tile([128, 65], F32, tag="av", padded_shape=[128, 512])
                    for st in range(SKT):
                        nc.tensor.matmul(
                            avp[:, 0:65],
                            e_list[st][:, j * CH + qb * 128: j * CH + (qb + 1) * 128],
                            vp[:, st * HPG * 65 + h * 65: st * HPG * 65 + (h + 1) * 65],
                            start=(st == 0),
                            stop=(st == SKT - 1),
                        )
                    # normalize: col 0 holds the softmax sum for each q row
                    rec = wrk.tile([128, 1], F32, tag="rec", bufs=4)
                    nc.vector.reciprocal(rec[:, :], avp[:, 0:1])
                    ot2 = wrk.tile([128, HD], F32, tag="ot2", bufs=4)
                    nc.vector.tensor_scalar_mul(ot2[:, :], avp[:, 1:65], rec[:, 0:1])

                    def tp_unit(j=j, qb=qb, ot2=ot2, p=p, qc=qc):
                        tpp = ps.tile([128, 512], F32, tag="acc", name=f"tp{p}{qc}{j}{qb}")
                        nc.tensor.transpose(tpp[0:HD, 0:128], ot2[:, :], ident[:, :])
                        nc.vector.tensor_copy(
                            ot[j * HD:(j + 1) * HD,
                               p * S + qc * CH + qb * 128: p * S + qc * CH + (qb + 1) * 128],
                            tpp[0:HD, 0:128],
                        )
                    tps.append(tp_unit)

                def unit(k):
                    if k < 8:
                        av_unit(k % 2, k // 2)
                    # run transposes two units behind their DVE normalize
                    if k >= 2:
                        tps.pop(0)()

                units.extend((lambda k=k: unit(k)) for k in range(10))
                return units

            # ---- per head-pair: Q^T, K^T then attention ----
            for qc in range(NQC):
                for w in (0, 1):
                    qk_chain(0, w, qc)

            carry = []        # deferred AV phase of the previous q-chunk
            carry_done = None  # (p, qc) whose ot rows complete when carry drains

            def drain_carry_step():
                if carry:
                    carry.pop(0)()
                    if not carry and carry_done is not None:
                        pp, cqc = carry_done
                        if pp == NPAIR - 1:
                            filler.extend(
                                (lambda qc=cqc, m=m: proj_chain(qc, m))
                                for m in range(NM)
                            )

            for p in range(NPAIR):
                qt, ktt = qk_tiles[p]
                if p + 1 < NPAIR:
                    filler.extend(
                        (lambda p=p, w=w, qc=qc: qk_chain(p + 1, w, qc))
                        for qc in range(NQC) for w in (0, 1)
                    )

                for qc in range(NQC):
                    fused = (CH == 512)
                    e_list = []
                    for st in range(SKT):
                        # V' production interleaved into the very first
                        # attention block (it must precede the first carry
                        # AV in PE program order; here it also overlaps exp).
                        if p == 0 and qc == 0:
                            v_chain(st)
                        e = ep.tile([128, 2 * CH], BF16, tag="e")
                        if fused:
                            sp = ps.tile([128, 2 * CH], F32, tag="sp", bufs=2)
                        for j in (0, 1):
                            hs = slice(j * 64, (j + 1) * 64)
                            if fused:
                                spv = sp[:, j * CH:(j + 1) * CH]
                            else:
                                spj = ps.tile([128, CH], F32, tag="sp", bufs=2)
                                spv = spj[:, :]
                            nc.tensor.matmul(
                                spv,
                                ktt[hs, st * 128:(st + 1) * 128],
                                qt[hs, qc * CH:(qc + 1) * CH],
                                start=True,
                                stop=True,
                                tile_position=(j * 64, 0),
                            )
                            if has_bias:
                                b8 = ep.tile([128, CH], F32, tag="b8", bufs=2)
                                nc.sync.dma_start(
                                    b8[:, :],
                                    bias8_dram[st * 128:(st + 1) * 128, qc * CH:(qc + 1) * CH],
                                )
                                nc.vector.tensor_add(spv, spv, b8[:, :])
                            if not fused:
                                nc.scalar.activation(
                                    e[:, j * CH:(j + 1) * CH], spv, AF.Exp, scale=0.125
                                )
                        if fused:
                            nc.scalar.activation(e[:, :], sp[:, :], AF.Exp, scale=0.125)
                        e_list.append(e)
                        # PE filler while exp trails: one deferred AV unit of
                        # the previous q-chunk per k-tile, then chains. In
                        # the last pair, pop chains alongside the carry too —
                        # the projection tiles queue up there and the PE (not
                        # ScalarE) is the engine that must never idle.
                        if not (p == 0 and qc == 0):
                            if carry:
                                drain_carry_step()
                                if p == NPAIR - 1 and st % 2 == 1:
                                    emit_filler(1)
                            else:
                                emit_filler(1)
                    # whatever's left of the previous chunk's AV phase
                    while carry:
                        drain_carry_step()
                    carry = make_carry(p, qc, e_list)
                    carry_done = (p, qc)

            # ---- tail: last q-chunk's AV phase + remaining projections ----
            while carry:
                drain_carry_step()
            emit_filler(len(filler))

    nc.compile()
    return nc


_BUILD_CACHE = {}


def _get_program(S, has_bqkv, has_bias):
    key = (S, has_bqkv, has_bias)
    if key not in _BUILD_CACHE:
        _BUILD_CACHE[key] = build_program(S, has_bqkv, has_bias)
    return _BUILD_CACHE[key]


def make_in_maps(query, bias, w_qkv, b_qkv, w_o, has_bqkv, has_bias):
    bf = ml_dtypes.bfloat16
    in_maps = []
    for c in range(NCORES):
        b, g = divmod(c, 2)
        cols = slice(g * GF, (g + 1) * GF)
        w_g = np.concatenate(
            [w_qkv[:, cols], w_qkv[:, HIDDEN:][:, cols], w_qkv[:, 2 * HIDDEN:][:, cols]],
            axis=1,
        )
        m = {
            "x": np.ascontiguousarray(query[b].T).astype(bf),
            "wqkv": np.ascontiguousarray(w_g).astype(bf),
            "wo": np.ascontiguousarray(w_o[cols]).astype(bf),
            "ident": np.eye(128, dtype=np.float32),
        }
        if has_bqkv:
            b_g = np.concatenate(
                [b_qkv[cols], b_qkv[HIDDEN:][cols], b_qkv[2 * HIDDEN:][cols]]
            )
            m["bqkv"] = b_g.reshape(1, 3 * GF).astype(bf)
        if has_bias:
            m["bias8t"] = np.ascontiguousarray(bias[0, 0].T * 8.0).astype(np.float32)
        in_maps.append(m)
    return in_maps


def assemble_output(results, b_o, S=S_FULL):
    NQC = S // min(512, S)
    out = np.zeros((B, S, HIDDEN), np.float32)
    for c in range(NCORES):
        b, _g = divmod(c, 2)
        y = results[c]["y"]  # [NM, NQC, 128, CH]
        yt = y.transpose(0, 2, 1, 3).reshape(HIDDEN, S)
        out[b] += yt.T
    out += np.asarray(b_o, np.float32)[None, None, :]
    return out


def kernel(query, bias, w_qkv, b_qkv, w_o, b_o, _trace=False):
    query = np.asarray(query, np.float32)
    bias = np.asarray(bias, np.float32)
    w_qkv = np.asarray(w_qkv, np.float32)
    b_qkv = np.asarray(b_qkv, np.float32)
    w_o = np.asarray(w_o, np.float32)
    b_o = np.asarray(b_o, np.float32)

    has_bqkv = bool(np.any(b_qkv))
    has_bias = bool(np.any(bias))
    nc = _get_program(S_FULL, has_bqkv, has_bias)
    in_maps = make_in_maps(query, bias, w_qkv, b_qkv, w_o, has_bqkv, has_bias)
    res = run_bass_kernel_spmd(
        nc, in_maps, core_ids=list(range(NCORES)), trace=_trace
    )
    out = assemble_output(res.results, b_o)
    if _trace:
        return out, res
    return out


# revision 20
# speedup vs baseline: 1.0401x; 1.0401x over previous
"""Multi-head attention (B=4, S=2048, H=1024, 16 heads) on 8 TRN2 NeuronCores.

Sharding: core c handles (batch b = c//2, head-group g = c%2 of 8 heads).
Per-core device program (activations feature-major / transposed):
  X^T  [1024,2048]  host-transposed bf16-cast query shard
  Q^T,K^T = Wq/k^T X^T            (PE, bf16, fp32 PSUM)
  V       = X W_v  (natural [s,d] via lhsT=X^T tiles), stored as
            V' = [1 | V] per head (leading ones column -> softmax sums)
  S^T  = per head, per k-tile: lhsT=K^T slice, rhs=Q^T slice
  E    = exp(S^T/8)  on ScalarE, bf16 out
  O'   = E^T V' per (head, q-block of 128): lhsT = E k-tile slices
         (LDWEIGHTS pipelines under the 65-wide matmuls: measured
         ~33ns/matmul vs 227ns for the [65, 512]-out orientation), so
         O'[q, 0] = softmax sum, O'[q, 1:65] = unnormalized O.
  normalize: per-partition reciprocal of col 0 + tensor_scalar_mul -> O[q,d]
  O^T  via PE transpose (identity matmul) -> assemble ot [d, q]
  Y^T  = W_o^T O^T   -> DMA out per [128,512] tile (split across 2 queues)
Host: gathers per-core Y^T tiles, sums the two head-group partials per batch,
adds b_o. b_qkv / attention bias are zero in this problem; nonzero values are
still handled (extra rank-1 bias matmuls / DVE bias adds) via build flags.

Scheduling: the PE is the critical engine; ScalarE exp (~285us) is a close
second. The AV phase of q-chunk qc is interleaved into the S/exp loop of the
NEXT q-chunk (one AV chain per k-tile iteration), and independent matmul
chains (V' tiles, next pair's Q/K projections, output-projection tiles) are
emitted as *filler* to keep the PE saturated while exp trails.
"""

import numpy as np
import ml_dtypes

import concourse.bass as bass
import concourse.tile as tile
from concourse import bacc
import concourse.mybir as mybir
from concourse.bass_utils import run_bass_kernel_spmd

F32 = mybir.dt.float32
BF16 = mybir.dt.bfloat16
AF = mybir.ActivationFunctionType

HIDDEN = 1024
HEADS = 16
HD = 64
B = 4
S_FULL = 2048
NCORES = 8
HPG = HEADS // 2          # heads per group/core = 8
GF = HPG * HD             # group feature width = 512
NPAIR = HPG // 2          # head pairs per core = 4


def build_program(S=S_FULL, has_bqkv=False, has_bias=False):
    KT = HIDDEN // 128            # hidden k-tiles = 8
    CH = min(512, S)              # free-dim chunk
    NQC = S // CH                 # q chunks
    SKT = S // 128                # seq k-tiles (attention contraction)
    NM = HIDDEN // 128            # output-projection m-tiles = 8
    NQB = CH // 128               # 128-row q-blocks per chunk = 4

    nc = bacc.Bacc(
        "TRN2",
        target_bir_lowering=False,
        debug=False,
        enable_asserts=False,
        num_devices=NCORES,
    )

    x_dram = nc.dram_tensor("x", [HIDDEN, S], BF16, kind="ExternalInput")  # X^T, host-transposed
    wqkv_dram = nc.dram_tensor("wqkv", [HIDDEN, 3 * GF], BF16, kind="ExternalInput")
    wo_dram = nc.dram_tensor("wo", [GF, HIDDEN], BF16, kind="ExternalInput")
    id_dram = nc.dram_tensor("ident", [128, 128], F32, kind="ExternalInput")
    if has_bqkv:
        bqkv_dram = nc.dram_tensor("bqkv", [1, 3 * GF], BF16, kind="ExternalInput")
    if has_bias:
        # host passes bias[0,0].T * 8 so exp(0.125*(S + bias8)) = exp(S/8 + bias)
        bias8_dram = nc.dram_tensor("bias8t", [S, S], F32, kind="ExternalInput")
    y_dram = nc.dram_tensor("y", [NM, NQC, 128, CH], BF16, kind="ExternalOutput")

    with tile.TileContext(nc) as tc:
        with (
            tc.tile_pool(name="res", bufs=1) as res,
            tc.tile_pool(name="wrk", bufs=2) as wrk,
            tc.tile_pool(name="ep", bufs=26) as ep,
            tc.tile_pool(name="ps", bufs=2, space="PSUM") as ps,
        ):
            xt = res.tile([128, KT * S], BF16, tag="xt")
            wqkv = res.tile([128, KT * 3 * GF], BF16, tag="wqkv")
            wo = res.tile([128, (GF // 128) * HIDDEN], BF16, tag="wo")
            ident = res.tile([128, 128], F32, tag="ident")
            # V' tiles: per s-tile block of 8 heads x 65 cols (col 0 = 1.0)
            vp = res.tile([128, SKT * HPG * 65], BF16, tag="vp")
            ot = res.tile([128, NPAIR * S], BF16, tag="ot")

            nc.vector.memset(vp[:, :], 1.0)

            # ---- input DMA, chunked so compute can start early ----
            # (fewer, wider DMAs empirically beat "priority" column slices)
            for kt in range(KT):
                nc.gpsimd.dma_start(
                    wqkv[:, kt * 3 * GF:(kt + 1) * 3 * GF],
                    wqkv_dram[kt * 128:(kt + 1) * 128, :],
                )
            for qc in range(NQC):
                for kt in range(KT):
                    eng = nc.sync if (kt % 2 == 0) else nc.gpsimd
                    eng.dma_start(
                        xt[:, kt * S + qc * CH: kt * S + (qc + 1) * CH],
                        x_dram[kt * 128:(kt + 1) * 128, qc * CH:(qc + 1) * CH],
                    )
            nc.sync.dma_start(ident[:, :], id_dram[:, :])
            for ft in range(GF // 128):
                nc.gpsimd.dma_start(
                    wo[:, ft * HIDDEN:(ft + 1) * HIDDEN],
                    wo_dram[ft * 128:(ft + 1) * 128, :],
                )
            if has_bqkv:
                bq = res.tile([1, 3 * GF], BF16, tag="bq")
                nc.sync.dma_start(bq[:, :], bqkv_dram[:, :])
                ones = res.tile([1, CH], BF16, tag="ones")
                nc.vector.memset(ones[:, :], 1.0)

            def acc_matmul(out_ps, lhsT_of, rhs_of, bias_lhsT, bias_rhs):
                """Accumulate KT matmuls (+ optional rank-1 bias term) into PSUM."""
                if bias_lhsT is not None:
                    nc.tensor.matmul(out_ps, bias_lhsT, bias_rhs, start=True, stop=False)
                for kt in range(KT):
                    nc.tensor.matmul(
                        out_ps,
                        lhsT_of(kt),
                        rhs_of(kt),
                        start=(kt == 0 and bias_lhsT is None),
                        stop=(kt == KT - 1),
                    )

            # ---- chain emitters (each ~1.7us of PE time) ----
            def v_chain(st):
                """V' for all 8 heads at s-tile st, natural [s, d] layout."""
                vps = ps.tile([128, GF], F32, tag="acc", padded_shape=[128, 512])
                acc_matmul(
                    vps[:, 0:GF],
                    lambda kt, st=st: xt[:, kt * S + st * 128: kt * S + (st + 1) * 128],
                    lambda kt: wqkv[:, kt * 3 * GF + 2 * GF: kt * 3 * GF + 3 * GF],
                    ones[0:1, 0:128] if has_bqkv else None,
                    bq[0:1, 2 * GF:3 * GF] if has_bqkv else None,
                )
                dst = vp[:, st * HPG * 65:(st + 1) * HPG * 65]
                dst = dst.rearrange("p (h c) -> p h c", c=65)[:, :, 1:65]
                src = vps[:, 0:GF].rearrange("p (h c) -> p h c", c=64)
                nc.vector.tensor_copy(dst, src)

            qk_tiles = {}  # p -> (qt, ktt)

            def qk_chain(p, which, qc):
                """One [128, CH] chunk of Q^T (which=0) or K^T (which=1) for pair p."""
                if p not in qk_tiles:
                    qk_tiles[p] = (
                        wrk.tile([128, S], BF16, tag="qt", name=f"qt{p}"),
                        wrk.tile([128, S], BF16, tag="ktt", name=f"ktt{p}"),
                    )
                dst_sb = qk_tiles[p][which]
                colbase = p * 128 + (GF if which else 0)
                qkps = ps.tile([128, CH], F32, tag="acc")
                acc_matmul(
                    qkps[:, :],
                    lambda kt, cb=colbase: wqkv[:, kt * 3 * GF + cb: kt * 3 * GF + cb + 128],
                    lambda kt, qc=qc: xt[:, kt * S + qc * CH: kt * S + (qc + 1) * CH],
                    bq[0:1, colbase:colbase + 128] if has_bqkv else None,
                    ones[0:1, 0:CH] if has_bqkv else None,
                )
                nc.vector.tensor_copy(dst_sb[:, qc * CH:(qc + 1) * CH], qkps[:, :])

            def proj_chain(qc, m):
                """Y^T tile [128, CH] at (hidden-slice m, q-chunk qc)."""
                yps = ps.tile([128, CH], F32, tag="acc")
                for ft in range(GF // 128):
                    nc.tensor.matmul(
                        yps[:, :],
                        wo[:, ft * HIDDEN + m * 128: ft * HIDDEN + (m + 1) * 128],
                        ot[:, ft * S + qc * CH: ft * S + (qc + 1) * CH],
                        start=(ft == 0),
                        stop=(ft == GF // 128 - 1),
                    )
                ysb = wrk.tile([128, CH], BF16, tag="ysb", bufs=4)
                nc.vector.tensor_copy(ysb[:, :], yps[:, :])
                # split the writeback across queues so the last tiles don't
                # extend the tail; the final q-chunk lands after the last exp,
                # so its tiles may also use the then-idle scalar queue.
                if qc == NQC - 1:
                    third = CH // 4
                    nc.sync.dma_start(y_dram[m, qc, :, 0:third], ysb[:, 0:third])
                    nc.gpsimd.dma_start(
                        y_dram[m, qc, :, third:2 * third], ysb[:, third:2 * third]
                    )
                    nc.scalar.dma_start(y_dram[m, qc, :, 2 * third:], ysb[:, 2 * third:])
                else:
                    half = CH // 2
                    nc.sync.dma_start(y_dram[m, qc, :, 0:half], ysb[:, 0:half])
                    nc.gpsimd.dma_start(y_dram[m, qc, :, half:CH], ysb[:, half:CH])

            # FIFO of pending filler chains, popped inside the attention loop
            filler = []

            def emit_filler(n=1):
                for _ in range(n):
                    if not filler:
                        return
                    filler.pop(0)()

            def make_carry(p, qc, e_list):
                """Deferred AV phase for (p, qc): 8 av-units + 2 transpose
                tails, each ~0.5-0.6us of PE. Interleaved into the NEXT
                q-chunk's S/exp loop (one unit per k-tile iteration)."""
                units = []
                tps = []  # deferred (transpose + copy) closures

                def av_unit(j, qb, p=p, qc=qc, e_list=e_list, tps=tps):
                    h = p * 2 + j
                    avp = ps.tile([128, 65], F32, tag="av", padded_shape=[128, 512])
                    for st in range(SKT):
                        nc.tensor.matmul(
                            avp[:, 0:65],
                            e_list[st][:, j * CH + qb * 128: j * CH + (qb + 1) * 128],
                            vp[:, st * HPG * 65 + h * 65: st * HPG * 65 + (h + 1) * 65],
                            start=(st == 0),
                            stop=(st == SKT - 1),
                        )
                    # normalize: col 0 holds the softmax sum for each q row
                    rec = wrk.tile([128, 1], F32, tag="rec", bufs=4)
                    nc.vector.reciprocal(rec[:, :], avp[:, 0:1])
                    ot2 = wrk.tile([128, HD], F32, tag="ot2", bufs=4)
                    nc.vector.tensor_scalar_mul(ot2[:, :], avp[:, 1:65], rec[:, 0:1])

                    def tp_unit(j=j, qb=qb, ot2=ot2, p=p, qc=qc):
                        tpp = ps.tile([128, 512], F32, tag="acc", name=f"tp{p}{qc}{j}{qb}")
                        nc.tensor.transpose(tpp[0:HD, 0:128], ot2[:, :], ident[:, :])
                        nc.vector.tensor_copy(
                            ot[j * HD:(j + 1) * HD,
                               p * S + qc * CH + qb * 128: p * S + qc * CH + (qb + 1) * 128],
                            tpp[0:HD, 0:128],
                        )
                    tps.append(tp_unit)

                def unit(k):
                    if k < 8:
                        av_unit(k % 2, k // 2)
                    # run transposes two units behind their DVE normalize
                    if k >= 2:
                        tps.pop(0)()

                units.extend((lambda k=k: unit(k)) for k in range(10))
                return units

            # ---- per head-pair: Q^T, K^T then attention ----
            for qc in range(NQC):
                for w in (0, 1):
                    qk_chain(0, w, qc)

            carry = []        # deferred AV phase of the previous q-chunk
            carry_done = None  # (p, qc) whose ot rows complete when carry drains

            def drain_carry_step():
                if carry:
                    carry.pop(0)()
                    if not carry and carry_done is not None:
                        pp, cqc = carry_done
                        if pp == NPAIR - 1:
                            filler.extend(
                                (lambda qc=cqc, m=m: proj_chain(qc, m))
                                for m in range(NM)
                            )

            for p in range(NPAIR):
                qt, ktt = qk_tiles[p]
                if p + 1 < NPAIR:
                    filler.extend(
                        (lambda p=p, w=w, qc=qc: qk_chain(p + 1, w, qc))
                        for qc in range(NQC) for w in (0, 1)
                    )

                for qc in range(NQC):
                    fused = (CH == 512)
                    e_list = []
                    for st in range(SKT):
                        # V' production interleaved into the very first
                        # attention block (it must precede the first carry
                        # AV in PE program order; here it also overlaps exp).
                        if p == 0 and qc == 0:
                            v_chain(st)
                        e = ep.tile([128, 2 * CH], BF16, tag="e")
                        if fused:
                            sp = ps.tile([128, 2 * CH], F32, tag="sp", bufs=2)
                        for j in (0, 1):
                            hs = slice(j * 64, (j + 1) * 64)
                            if fused:
                                spv = sp[:, j * CH:(j + 1) * CH]
                            else:
                                spj = ps.tile([128, CH], F32, tag="sp", bufs=2)
                                spv = spj[:, :]
                            nc.tensor.matmul(
                                spv,
                                ktt[hs, st * 128:(st + 1) * 128],
                                qt[hs, qc * CH:(qc + 1) * CH],
                                start=True,
                                stop=True,
                                tile_position=(j * 64, 0),
                            )
                            if has_bias:
                                b8 = ep.tile([128, CH], F32, tag="b8", bufs=2)
                                nc.sync.dma_start(
                                    b8[:, :],
                                    bias8_dram[st * 128:(st + 1) * 128, qc * CH:(qc + 1) * CH],
                                )
                                nc.vector.tensor_add(spv, spv, b8[:, :])
                            if not fused:
                                nc.scalar.activation(
                                    e[:, j * CH:(j + 1) * CH], spv, AF.Exp, scale=0.125
                                )
                        if fused:
                            nc.scalar.activation(e[:, :], sp[:, :], AF.Exp, scale=0.125)
                        e_list.append(e)
                        # PE filler while exp trails: one deferred AV unit of
                        # the previous q-chunk per k-tile, then chains. In
                        # the last pair, pop chains alongside the carry too —
                        # the projection tiles queue up there and the PE (not
                        # ScalarE) is the engine that must never idle.
                        if not (p == 0 and qc == 0):
                            if carry:
                                drain_carry_step()
                            if st % 2 == 1 and (p == NPAIR - 1 or not carry):
                                emit_filler(1)
                    # whatever's left of the previous chunk's AV phase
                    while carry:
                        drain_carry_step()
                    carry = make_carry(p, qc, e_list)
                    carry_done = (p, qc)

            # ---- tail: last q-chunk's AV phase + remaining projections ----
            while carry:
                drain_carry_step()
            emit_filler(len(filler))

    nc.compile()
    return nc


_BUILD_CACHE = {}


def _get_program(S, has_bqkv, has_bias):
    key = (S, has_bqkv, has_bias)
    if key not in _BUILD_CACHE:
        _BUILD_CACHE[key] = build_program(S, has_bqkv, has_bias)
    return _BUILD_CACHE[key]


def make_in_maps(query, bias, w_qkv, b_qkv, w_o, has_bqkv, has_bias):
    bf = ml_dtypes.bfloat16
    in_maps = []
    for c in range(NCORES):
        b, g = divmod(c, 2)
        cols = slice(g * GF, (g + 1) * GF)
        w_g = np.concatenate(
            [w_qkv[:, cols], w_qkv[:, HIDDEN:][:, cols], w_qkv[:, 2 * HIDDEN:][:, cols]],
            axis=1,
        )
        m = {
            "x": np.ascontiguousarray(query[b].T).astype(bf),
            "wqkv": np.ascontiguousarray(w_g).astype(bf),
            "wo": np.ascontiguousarray(w_o[cols]).astype(bf),
            "ident": np.eye(128, dtype=np.float32),
        }
        if has_bqkv:
            b_g = np.concatenate(
                [b_qkv[cols], b_qkv[HIDDEN:][cols], b_qkv[2 * HIDDEN:][cols]]
            )
            m["bqkv"] = b_g.reshape(1, 3 * GF).astype(bf)
        if has_bias:
            m["bias8t"] = np.ascontiguousarray(bias[0, 0].T * 8.0).astype(np.float32)
        in_maps.append(m)
    return in_maps


def assemble_output(results, b_o, S=S_FULL):
    NQC = S // min(512, S)
    out = np.zeros((B, S, HIDDEN), np.float32)
    for c in range(NCORES):
        b, _g = divmod(c, 2)
        y = results[c]["y"]  # [NM, NQC, 128, CH]
        yt = y.transpose(0, 2, 1, 3).reshape(HIDDEN, S)
        out[b] += yt.T
    out += np.asarray(b_o, np.float32)[None, None, :]
    return out


def kernel(query, bias, w_qkv, b_qkv, w_o, b_o, _trace=False):
    query = np.asarray(query, np.float32)
    bias = np.asarray(bias, np.float32)
    w_qkv = np.asarray(w_qkv, np.float32)
    b_qkv = np.asarray(b_qkv, np.float32)
    w_o = np.asarray(w_o, np.float32)
    b_o = np.asarray(b_o, np.float32)

    has_bqkv = bool(np.any(b_qkv))
    has_bias = bool(np.any(bias))
    nc = _get_program(S_FULL, has_bqkv, has_bias)
    in_maps = make_in_maps(query, bias, w_qkv, b_qkv, w_o, has_bqkv, has_bias)
    res = run_bass_kernel_spmd(
        nc, in_maps, core_ids=list(range(NCORES)), trace=_trace
    )
    out = assemble_output(res.results, b_o)
    if _trace:
        return out, res
    return out


# revision 21
# speedup vs baseline: 1.0404x; 1.0003x over previous
"""Multi-head attention (B=4, S=2048, H=1024, 16 heads) on 8 TRN2 NeuronCores.

Sharding: core c handles (batch b = c//2, head-group g = c%2 of 8 heads).
Per-core device program (activations feature-major / transposed):
  X^T  [1024,2048]  host-transposed bf16-cast query shard
  Q^T,K^T = Wq/k^T X^T            (PE, bf16, fp32 PSUM)
  V       = X W_v  (natural [s,d] via lhsT=X^T tiles), stored as
            V' = [1 | V] per head (leading ones column -> softmax sums)
  S^T  = per head, per k-tile: lhsT=K^T slice, rhs=Q^T slice
  E    = exp(S^T/8)  on ScalarE, bf16 out
  O'   = E^T V' per (head, q-block of 128): lhsT = E k-tile slices
         (LDWEIGHTS pipelines under the 65-wide matmuls: measured
         ~33ns/matmul vs 227ns for the [65, 512]-out orientation), so
         O'[q, 0] = softmax sum, O'[q, 1:65] = unnormalized O.
  normalize: per-partition reciprocal of col 0 + tensor_scalar_mul -> O[q,d]
  O^T  via PE transpose (identity matmul) -> assemble ot [d, q]
  Y^T  = W_o^T O^T   -> DMA out per [128,512] tile (split across 2 queues)
Host: gathers per-core Y^T tiles, sums the two head-group partials per batch,
adds b_o. b_qkv / attention bias are zero in this problem; nonzero values are
still handled (extra rank-1 bias matmuls / DVE bias adds) via build flags.

Scheduling: the PE is the critical engine; ScalarE exp (~285us) is a close
second. The AV phase of q-chunk qc is interleaved into the S/exp loop of the
NEXT q-chunk (one AV chain per k-tile iteration), and independent matmul
chains (V' tiles, next pair's Q/K projections, output-projection tiles) are
emitted as *filler* to keep the PE saturated while exp trails.
"""

import numpy as np
import ml_dtypes

import concourse.bass as bass
import concourse.tile as tile
from concourse import bacc
import concourse.mybir as mybir
from concourse.bass_utils import run_bass_kernel_spmd

F32 = mybir.dt.float32
BF16 = mybir.dt.bfloat16
AF = mybir.ActivationFunctionType

HIDDEN = 1024
HEADS = 16
HD = 64
B = 4
S_FULL = 2048
NCORES = 8
HPG = HEADS // 2          # heads per group/core = 8
GF = HPG * HD             # group feature width = 512
NPAIR = HPG // 2          # head pairs per core = 4


def build_program(S=S_FULL, has_bqkv=False, has_bias=False):
    KT = HIDDEN // 128            # hidden k-tiles = 8
    CH = min(512, S)              # free-dim chunk
    NQC = S // CH                 # q chunks
    SKT = S // 128                # seq k-tiles (attention contraction)
    NM = HIDDEN // 128            # output-projection m-tiles = 8
    NQB = CH // 128               # 128-row q-blocks per chunk = 4

    nc = bacc.Bacc(
        "TRN2",
        target_bir_lowering=False,
        debug=False,
        enable_asserts=False,
        num_devices=NCORES,
    )

    x_dram = nc.dram_tensor("x", [HIDDEN, S], BF16, kind="ExternalInput")  # X^T, host-transposed
    wqkv_dram = nc.dram_tensor("wqkv", [HIDDEN, 3 * GF], BF16, kind="ExternalInput")
    wo_dram = nc.dram_tensor("wo", [GF, HIDDEN], BF16, kind="ExternalInput")
    id_dram = nc.dram_tensor("ident", [128, 128], F32, kind="ExternalInput")
    if has_bqkv:
        bqkv_dram = nc.dram_tensor("bqkv", [1, 3 * GF], BF16, kind="ExternalInput")
    if has_bias:
        # host passes bias[0,0].T * 8 so exp(0.125*(S + bias8)) = exp(S/8 + bias)
        bias8_dram = nc.dram_tensor("bias8t", [S, S], F32, kind="ExternalInput")
    y_dram = nc.dram_tensor("y", [NM, NQC, 128, CH], BF16, kind="ExternalOutput")

    with tile.TileContext(nc) as tc:
        with (
            tc.tile_pool(name="res", bufs=1) as res,
            tc.tile_pool(name="wrk", bufs=2) as wrk,
            tc.tile_pool(name="ep", bufs=26) as ep,
            tc.tile_pool(name="ps", bufs=2, space="PSUM") as ps,
        ):
            xt = res.tile([128, KT * S], BF16, tag="xt")
            wqkv = res.tile([128, KT * 3 * GF], BF16, tag="wqkv")
            wo = res.tile([128, (GF // 128) * HIDDEN], BF16, tag="wo")
            ident = res.tile([128, 128], F32, tag="ident")
            # V' tiles: per s-tile block of 8 heads x 65 cols (col 0 = 1.0)
            vp = res.tile([128, SKT * HPG * 65], BF16, tag="vp")
            ot = res.tile([128, NPAIR * S], BF16, tag="ot")

            nc.vector.memset(vp[:, :], 1.0)

            # ---- input DMA, chunked so compute can start early ----
            # (fewer, wider DMAs empirically beat "priority" column slices)
            for kt in range(KT):
                eng = nc.gpsimd if (kt % 2 == 0) else nc.sync
                eng.dma_start(
                    wqkv[:, kt * 3 * GF:(kt + 1) * 3 * GF],
                    wqkv_dram[kt * 128:(kt + 1) * 128, :],
                )
            for qc in range(NQC):
                for kt in range(KT):
                    eng = nc.sync if (kt % 2 == 0) else nc.gpsimd
                    eng.dma_start(
                        xt[:, kt * S + qc * CH: kt * S + (qc + 1) * CH],
                        x_dram[kt * 128:(kt + 1) * 128, qc * CH:(qc + 1) * CH],
                    )
            nc.sync.dma_start(ident[:, :], id_dram[:, :])
            for ft in range(GF // 128):
                eng = nc.gpsimd if (ft % 2 == 0) else nc.sync
                eng.dma_start(
                    wo[:, ft * HIDDEN:(ft + 1) * HIDDEN],
                    wo_dram[ft * 128:(ft + 1) * 128, :],
                )
            if has_bqkv:
                bq = res.tile([1, 3 * GF], BF16, tag="bq")
                nc.sync.dma_start(bq[:, :], bqkv_dram[:, :])
                ones = res.tile([1, CH], BF16, tag="ones")
                nc.vector.memset(ones[:, :], 1.0)

            def acc_matmul(out_ps, lhsT_of, rhs_of, bias_lhsT, bias_rhs):
                """Accumulate KT matmuls (+ optional rank-1 bias term) into PSUM."""
                if bias_lhsT is not None:
                    nc.tensor.matmul(out_ps, bias_lhsT, bias_rhs, start=True, stop=False)
                for kt in range(KT):
                    nc.tensor.matmul(
                        out_ps,
                        lhsT_of(kt),
                        rhs_of(kt),
                        start=(kt == 0 and bias_lhsT is None),
                        stop=(kt == KT - 1),
                    )

            # ---- chain emitters (each ~1.7us of PE time) ----
            def v_chain(st):
                """V' for all 8 heads at s-tile st, natural [s, d] layout."""
                vps = ps.tile([128, GF], F32, tag="acc", padded_shape=[128, 512])
                acc_matmul(
                    vps[:, 0:GF],
                    lambda kt, st=st: xt[:, kt * S + st * 128: kt * S + (st + 1) * 128],
                    lambda kt: wqkv[:, kt * 3 * GF + 2 * GF: kt * 3 * GF + 3 * GF],
                    ones[0:1, 0:128] if has_bqkv else None,
                    bq[0:1, 2 * GF:3 * GF] if has_bqkv else None,
                )
                dst = vp[:, st * HPG * 65:(st + 1) * HPG * 65]
                dst = dst.rearrange("p (h c) -> p h c", c=65)[:, :, 1:65]
                src = vps[:, 0:GF].rearrange("p (h c) -> p h c", c=64)
                nc.vector.tensor_copy(dst, src)

            qk_tiles = {}  # p -> (qt, ktt)

            def qk_chain(p, which, qc):
                """One [128, CH] chunk of Q^T (which=0) or K^T (which=1) for pair p."""
                if p not in qk_tiles:
                    qk_tiles[p] = (
                        wrk.tile([128, S], BF16, tag="qt", name=f"qt{p}"),
                        wrk.tile([128, S], BF16, tag="ktt", name=f"ktt{p}"),
                    )
                dst_sb = qk_tiles[p][which]
                colbase = p * 128 + (GF if which else 0)
                qkps = ps.tile([128, CH], F32, tag="acc")
                acc_matmul(
                    qkps[:, :],
                    lambda kt, cb=colbase: wqkv[:, kt * 3 * GF + cb: kt * 3 * GF + cb + 128],
                    lambda kt, qc=qc: xt[:, kt * S + qc * CH: kt * S + (qc + 1) * CH],
                    bq[0:1, colbase:colbase + 128] if has_bqkv else None,
                    ones[0:1, 0:CH] if has_bqkv else None,
                )
                nc.vector.tensor_copy(dst_sb[:, qc * CH:(qc + 1) * CH], qkps[:, :])

            def proj_chain(qc, m):
                """Y^T tile [128, CH] at (hidden-slice m, q-chunk qc)."""
                yps = ps.tile([128, CH], F32, tag="acc")
                for ft in range(GF // 128):
                    nc.tensor.matmul(
                        yps[:, :],
                        wo[:, ft * HIDDEN + m * 128: ft * HIDDEN + (m + 1) * 128],
                        ot[:, ft * S + qc * CH: ft * S + (qc + 1) * CH],
                        start=(ft == 0),
                        stop=(ft == GF // 128 - 1),
                    )
                ysb = wrk.tile([128, CH], BF16, tag="ysb", bufs=4)
                nc.vector.tensor_copy(ysb[:, :], yps[:, :])
                # split the writeback across queues so the last tiles don't
                # extend the tail; the final q-chunk lands after the last exp,
                # so its tiles may also use the then-idle scalar queue.
                if qc == NQC - 1:
                    third = CH // 4
                    nc.sync.dma_start(y_dram[m, qc, :, 0:third], ysb[:, 0:third])
                    nc.gpsimd.dma_start(
                        y_dram[m, qc, :, third:2 * third], ysb[:, third:2 * third]
                    )
                    nc.scalar.dma_start(y_dram[m, qc, :, 2 * third:], ysb[:, 2 * third:])
                else:
                    half = CH // 2
                    nc.sync.dma_start(y_dram[m, qc, :, 0:half], ysb[:, 0:half])
                    nc.gpsimd.dma_start(y_dram[m, qc, :, half:CH], ysb[:, half:CH])

            # FIFO of pending filler chains, popped inside the attention loop
            filler = []

            def emit_filler(n=1):
                for _ in range(n):
                    if not filler:
                        return
                    filler.pop(0)()

            def make_carry(p, qc, e_list):
                """Deferred AV phase for (p, qc): 8 av-units + 2 transpose
                tails, each ~0.5-0.6us of PE. Interleaved into the NEXT
                q-chunk's S/exp loop (one unit per k-tile iteration)."""
                units = []
                tps = []  # deferred (transpose + copy) closures

                def av_unit(j, qb, p=p, qc=qc, e_list=e_list, tps=tps):
                    h = p * 2 + j
                    avp = ps.tile([128, 65], F32, tag="av", padded_shape=[128, 512])
                    for st in range(SKT):
                        nc.tensor.matmul(
                            avp[:, 0:65],
                            e_list[st][:, j * CH + qb * 128: j * CH + (qb + 1) * 128],
                            vp[:, st * HPG * 65 + h * 65: st * HPG * 65 + (h + 1) * 65],
                            start=(st == 0),
                            stop=(st == SKT - 1),
                        )
                    # normalize: col 0 holds the softmax sum for each q row
                    rec = wrk.tile([128, 1], F32, tag="rec", bufs=4)
                    nc.vector.reciprocal(rec[:, :], avp[:, 0:1])
                    ot2 = wrk.tile([128, HD], F32, tag="ot2", bufs=4)
                    nc.vector.tensor_scalar_mul(ot2[:, :], avp[:, 1:65], rec[:, 0:1])

                    def tp_unit(j=j, qb=qb, ot2=ot2, p=p, qc=qc):
                        tpp = ps.tile([128, 512], F32, tag="acc", name=f"tp{p}{qc}{j}{qb}")
                        nc.tensor.transpose(tpp[0:HD, 0:128], ot2[:, :], ident[:, :])
                        nc.vector.tensor_copy(
                            ot[j * HD:(j + 1) * HD,
                               p * S + qc * CH + qb * 128: p * S + qc * CH + (qb + 1) * 128],
                            tpp[0:HD, 0:128],
                        )
                    tps.append(tp_unit)

                def unit(k):
                    if k < 8:
                        av_unit(k % 2, k // 2)
                    # run transposes two units behind their DVE normalize
                    if k >= 2:
                        tps.pop(0)()

                units.extend((lambda k=k: unit(k)) for k in range(10))
                return units

            # ---- per head-pair: Q^T, K^T then attention ----
            for qc in range(NQC):
                for w in (0, 1):
                    qk_chain(0, w, qc)

            carry = []        # deferred AV phase of the previous q-chunk
            carry_done = None  # (p, qc) whose ot rows complete when carry drains

            def drain_carry_step():
                if carry:
                    carry.pop(0)()
                    if not carry and carry_done is not None:
                        pp, cqc = carry_done
                        if pp == NPAIR - 1:
                            filler.extend(
                                (lambda qc=cqc, m=m: proj_chain(qc, m))
                                for m in range(NM)
                            )

            for p in range(NPAIR):
                qt, ktt = qk_tiles[p]
                if p + 1 < NPAIR:
                    filler.extend(
                        (lambda p=p, w=w, qc=qc: qk_chain(p + 1, w, qc))
                        for qc in range(NQC) for w in (0, 1)
                    )

                for qc in range(NQC):
                    fused = (CH == 512)
                    e_list = []
                    for st in range(SKT):
                        # V' production interleaved into the very first
                        # attention block (it must precede the first carry
                        # AV in PE program order; here it also overlaps exp).
                        if p == 0 and qc == 0:
                            v_chain(st)
                        e = ep.tile([128, 2 * CH], BF16, tag="e")
                        if fused:
                            sp = ps.tile([128, 2 * CH], F32, tag="sp", bufs=2)
                        for j in (0, 1):
                            hs = slice(j * 64, (j + 1) * 64)
                            if fused:
                                spv = sp[:, j * CH:(j + 1) * CH]
                            else:
                                spj = ps.tile([128, CH], F32, tag="sp", bufs=2)
                                spv = spj[:, :]
                            nc.tensor.matmul(
                                spv,
                                ktt[hs, st * 128:(st + 1) * 128],
                                qt[hs, qc * CH:(qc + 1) * CH],
                                start=True,
                                stop=True,
                                tile_position=(j * 64, 0),
                            )
                            if has_bias:
                                b8 = ep.tile([128, CH], F32, tag="b8", bufs=2)
                                nc.sync.dma_start(
                                    b8[:, :],
                                    bias8_dram[st * 128:(st + 1) * 128, qc * CH:(qc + 1) * CH],
                                )
                                nc.vector.tensor_add(spv, spv, b8[:, :])
                            if not fused:
                                nc.scalar.activation(
                                    e[:, j * CH:(j + 1) * CH], spv, AF.Exp, scale=0.125
                                )
                        if fused:
                            nc.scalar.activation(e[:, :], sp[:, :], AF.Exp, scale=0.125)
                        e_list.append(e)
                        # PE filler while exp trails: one deferred AV unit of
                        # the previous q-chunk per k-tile, then chains. In
                        # the last pair, pop chains alongside the carry too —
                        # the projection tiles queue up there and the PE (not
                        # ScalarE) is the engine that must never idle.
                        if not (p == 0 and qc == 0):
                            if carry:
                                drain_carry_step()
                            if st % 2 == 1 and (p == NPAIR - 1 or not carry):
                                emit_filler(1)
                    # whatever's left of the previous chunk's AV phase
                    while carry:
                        drain_carry_step()
                    carry = make_carry(p, qc, e_list)
                    carry_done = (p, qc)

            # ---- tail: last q-chunk's AV phase + remaining projections ----
            while carry:
                drain_carry_step()
            emit_filler(len(filler))

    nc.compile()
    return nc


_BUILD_CACHE = {}


def _get_program(S, has_bqkv, has_bias):
    key = (S, has_bqkv, has_bias)
    if key not in _BUILD_CACHE:
        _BUILD_CACHE[key] = build_program(S, has_bqkv, has_bias)
    return _BUILD_CACHE[key]


def make_in_maps(query, bias, w_qkv, b_qkv, w_o, has_bqkv, has_bias):
    bf = ml_dtypes.bfloat16
    in_maps = []
    for c in range(NCORES):
        b, g = divmod(c, 2)
        cols = slice(g * GF, (g + 1) * GF)
        w_g = np.concatenate(
            [w_qkv[:, cols], w_qkv[:, HIDDEN:][:, cols], w_qkv[:, 2 * HIDDEN:][:, cols]],
            axis=1,
        )
        m = {
            "x": np.ascontiguousarray(query[b].T).astype(bf),
            "wqkv": np.ascontiguousarray(w_g).astype(bf),
            "wo": np.ascontiguousarray(w_o[cols]).astype(bf),
            "ident": np.eye(128, dtype=np.float32),
        }
        if has_bqkv:
            b_g = np.concatenate(
                [b_qkv[cols], b_qkv[HIDDEN:][cols], b_qkv[2 * HIDDEN:][cols]]
            )
            m["bqkv"] = b_g.reshape(1, 3 * GF).astype(bf)
        if has_bias:
            m["bias8t"] = np.ascontiguousarray(bias[0, 0].T * 8.0).astype(np.float32)
        in_maps.append(m)
    return in_maps


def assemble_output(results, b_o, S=S_FULL):
    NQC = S // min(512, S)
    out = np.zeros((B, S, HIDDEN), np.float32)
    for c in range(NCORES):
        b, _g = divmod(c, 2)
        y = results[c]["y"]  # [NM, NQC, 128, CH]
        yt = y.transpose(0, 2, 1, 3).reshape(HIDDEN, S)
        out[b] += yt.T
    out += np.asarray(b_o, np.float32)[None, None, :]
    return out


def kernel(query, bias, w_qkv, b_qkv, w_o, b_o, _trace=False):
    query = np.asarray(query, np.float32)
    bias = np.asarray(bias, np.float32)
    w_qkv = np.asarray(w_qkv, np.float32)
    b_qkv = np.asarray(b_qkv, np.float32)
    w_o = np.asarray(w_o, np.float32)
    b_o = np.asarray(b_o, np.float32)

    has_bqkv = bool(np.any(b_qkv))
    has_bias = bool(np.any(bias))
    nc = _get_program(S_FULL, has_bqkv, has_bias)
    in_maps = make_in_maps(query, bias, w_qkv, b_qkv, w_o, has_bqkv, has_bias)
    res = run_bass_kernel_spmd(
        nc, in_maps, core_ids=list(range(NCORES)), trace=_trace
    )
    out = assemble_output(res.results, b_o)
    if _trace:
        return out, res
    return out
